# revision 33
# baseline (speedup 1.0000x reference)
"""Multi-head attention (B=4, S=1024, D=1024, H=16) on 8 Trainium2 NeuronCores.

Sharding (tensor-parallel over heads x data-parallel over batch):
core c handles batch b=c//2 and head-half hh=c%2 (8 of the 16 heads).
Each core projects Q/K/V for its 8 heads only (zero duplicated
projection FLOPs), runs attention for its 8 heads over the full
1024x1024 score matrix, and computes the PARTIAL output projection
ctx_half @ Wo[:, half].T.  The host sums the two partials per batch and
adds bo during the gather -- no device collectives.

Performance notes:
 - All device DMA is contiguous and coarse.  Tiny strided DMAs (ones
   columns, per-partition bias/mask columns) are eliminated: they
   generate thousands of 4-byte packets that monopolize the shared DMA
   engines for ~80us.  Mask/bias columns arrive host-packed as [128,n]
   tiles; the softmax ones-columns are written by a DVE tensor_scalar.
 - x and Wq/Wk/Wv stream in as bf16 (half the DMA bytes); everything
   downstream of the projections stays float32r (same 1 column/cycle PE
   streaming rate as bf16 at N=512, full fp32 precision).
 - Softmax reciprocal via reciprocal_approx_fast (~5x faster than the
   multipass DVE reciprocal).
 - Output-projection PSUM evacuation on the Scalar engine (idle after
   attention; DVE is the busier engine).
"""

import sys

for _p in ("/opt/trn_rl_repo", "/opt/pypackages"):
    if _p not in sys.path:
        sys.path.append(_p)

import numpy as np

B = 4
S = 1024
D = 1024
H = 16
HD = 64
HC = 8            # heads per core
DC = HC * HD      # 512 projection dims per core
KT = D // 128     # 8 contraction tiles (model dim)
CT = DC // 128    # 4 contraction tiles (core's ctx dims) = head pairs
SKT = S // 128    # 8 key tiles
NCORES = 8

_COMPILED = None


def _build():
    import concourse.bass as bass
    import concourse.mybir as mybir
    from concourse import bacc
    from concourse.bass import ts
    from concourse.tile import TileContext

    f32 = mybir.dt.float32
    f32r = mybir.dt.float32r
    bf16 = mybir.dt.bfloat16
    EXP = mybir.ActivationFunctionType.Exp

    nc = bacc.Bacc("TRN2", target_bir_lowering=False, debug=False,
                   num_devices=NCORES)

    # activations/weights pre-transposed + contiguous; x and W in bf16
    xq_d = nc.dram_tensor("xq", [D, S], bf16, kind="ExternalInput")
    xk_d = nc.dram_tensor("xk", [D, S], bf16, kind="ExternalInput")
    xv_d = nc.dram_tensor("xv", [D, S], bf16, kind="ExternalInput")
    wq_d = nc.dram_tensor("Wq", [D, DC], bf16, kind="ExternalInput")
    wk_d = nc.dram_tensor("Wk", [D, DC], bf16, kind="ExternalInput")
    wv_d = nc.dram_tensor("Wv", [D, DC], bf16, kind="ExternalInput")
    wo_d = nc.dram_tensor("Wo", [DC, D], f32, kind="ExternalInput")
    # host-packed per-partition columns: mask bias [128,SKT], biases [128,CT]
    mb_d = nc.dram_tensor("maskbias", [128, SKT], f32, kind="ExternalInput")
    bq_d = nc.dram_tensor("bqc", [128, CT], f32, kind="ExternalInput")
    bk_d = nc.dram_tensor("bkc", [128, CT], f32, kind="ExternalInput")
    bv_d = nc.dram_tensor("bv", [DC], f32, kind="ExternalInput")
    out_d = nc.dram_tensor("out", [D, S], f32, kind="ExternalOutput")

    with TileContext(nc) as tc:
        from contextlib import ExitStack
        with ExitStack() as stack:
            const = stack.enter_context(tc.tile_pool(name="const", bufs=1))
            vnat_p = stack.enter_context(tc.tile_pool(name="vnat", bufs=1))
            ctx_p = stack.enter_context(tc.tile_pool(name="ctxT", bufs=1))
            xq_p = stack.enter_context(tc.tile_pool(name="xq", bufs=1))
            xk_p = stack.enter_context(tc.tile_pool(name="xk", bufs=1))
            xv_p = stack.enter_context(tc.tile_pool(name="xv", bufs=1))
            w_p = stack.enter_context(tc.tile_pool(name="w", bufs=1))
            proj_ps = stack.enter_context(
                tc.tile_pool(name="proj_ps", bufs=2, space="PSUM"))
            scores_ps = stack.enter_context(
                tc.tile_pool(name="scores_ps", bufs=2, space="PSUM"))
            ctx_ps = stack.enter_context(
                tc.tile_pool(name="ctx_ps", bufs=1, space="PSUM"))

            # ---- small host-packed constants first (complete in ~1us,
            # unblock the projection bias-adds) ------------------------------
            mb_sb = const.tile([128, SKT], f32, tag="mb")
            nc.sync.dma_start(mb_sb[:], mb_d[:])
            maskb = [mb_sb[:, t:t + 1] for t in range(SKT)]
            bqc = const.tile([128, CT], f32, tag="bqc")
            nc.sync.dma_start(bqc[:], bq_d[:])
            bkc = const.tile([128, CT], f32, tag="bkc")
            nc.sync.dma_start(bkc[:], bk_d[:])
            bv_bc = const.tile([128, DC], f32, tag="bvbc")
            nc.sync.dma_start(
                bv_bc[:],
                bass.AP(tensor=bv_d, offset=0, ap=[[0, 128], [1, DC]]))

            # ---- big streaming loads, in consumption order.  V/K inputs
            # stream on the sync HWDGE queues, Q/O inputs on the gpsimd
            # SWDGE queues (separate DMA engines -> parallel streams).
            # Weight/activation k-tiles interleave so accumulation chain
            # k can start as soon as pair k has landed. -------------------
            xv_t, wv_t, xk_t, wk_t, xq_t, wq_t = [], [], [], [], [], []
            # two independent DMA streams (sync HWDGE / gpsimd SWDGE),
            # alternating per k.  K/Q si-half-0 inputs lead so attention
            # can start ~20us in; V inputs follow (their chains weave into
            # the first attention block); si-half-1 and Wo trail.
            eng = [nc.sync, nc.gpsimd]
            for k in range(KT):
                e = eng[k % 2]
                t = w_p.tile([128, DC], bf16, tag=f"wk{k}")
                if k < 2:
                    e.dma_start(t[:, 0:256], wk_d[ts(k, 128), 0:256])
                    e.dma_start(t[:, 256:512], wk_d[ts(k, 128), 256:512])
                else:
                    e.dma_start(t[:], wk_d[ts(k, 128), :])
                wk_t.append(t)
                t = xk_p.tile([128, S], bf16, tag=f"xk{k}")
                e.dma_start(t[:, 0:512], xk_d[ts(k, 128), 0:512])
                xk_t.append(t)
            for k in range(KT):
                t = w_p.tile([128, DC], bf16, tag=f"wv{k}")
                eng[k % 2].dma_start(t[:], wv_d[ts(k, 128), :])
                wv_t.append(t)
                t = xv_p.tile([128, S], bf16, tag=f"xv{k}")
                eng[k % 2].dma_start(t[:, 0:512], xv_d[ts(k, 128), 0:512])
                xv_t.append(t)
            for k in range(KT):
                e = eng[k % 2]
                t = w_p.tile([128, DC], bf16, tag=f"wq{k}")
                e.dma_start(t[:], wq_d[ts(k, 128), :])
                wq_t.append(t)
                t = xq_p.tile([128, S], bf16, tag=f"xq{k}")
                e.dma_start(t[:, 0:512], xq_d[ts(k, 128), 0:512])
                xq_t.append(t)
            for k in range(KT):
                eng[k % 2].dma_start(xv_t[k][:, 512:1024],
                                     xv_d[ts(k, 128), 512:1024])
            for k in range(KT):
                eng[k % 2].dma_start(xk_t[k][:, 512:1024],
                                     xk_d[ts(k, 128), 512:1024])
            for k in range(KT):
                eng[k % 2].dma_start(xq_t[k][:, 512:1024],
                                     xq_d[ts(k, 128), 512:1024])
            wo_t = []
            for k in range(CT):
                t = w_p.tile([128, D], f32r, tag=f"wo{k}")
                eng[k % 2].dma_start(t[:], wo_d[ts(k, 128), :].bitcast(f32r))
                wo_t.append(t)

            # preload the exp activation table during the startup window so
            # the first real exp doesn't pay the ~2.7us table switch
            warm = const.tile([128, 1], f32, tag="warm")
            nc.scalar.activation(warm[:], mb_sb[:, 0:1], EXP, scale=1.0)

            # ---- V projection (natural layout: [sk, dout] + ones col) ------
            # chains are issued inside the first attention block (weave);
            # only the ones columns are written up front
            vnat = [vnat_p.tile([128, HC * 65], f32r, tag=f"v{m}",
                                name=f"vnat{m}")
                    for m in range(SKT)]
            bv3 = bv_bc[:].rearrange("p (a b) -> p a b", b=1)
            for m in range(SKT):
                vv = vnat[m][:].rearrange("p (h x) -> p h x", x=65)
                # ones column: in*0 + 1 from a known-finite source (DVE
                # rounds to f32r; memset can't produce f32r)
                nc.vector.tensor_scalar(vv[:, :, 64:65], bv3[:, 0:HC, :],
                                        0.0, 1.0,
                                        mybir.AluOpType.mult,
                                        mybir.AluOpType.add)

            def vchain(m):
                vv = vnat[m][:].rearrange("p (h x) -> p h x", x=65)
                ps = proj_ps.tile([128, 512], f32, tag="pp")
                for k in range(KT):
                    nc.tensor.matmul(
                        ps[:], xv_t[k][:, ts(m, 128)], wv_t[k][:],
                        start=(k == 0), stop=(k == KT - 1))
                nc.vector.tensor_add(
                    vv[:, :, 0:64],
                    ps[:].rearrange("p (h x) -> p h x", x=64),
                    bv_bc[:].rearrange("p (h x) -> p h x", x=64))

            # ---- per head-pair: K/Q projection + attention -----------------
            ctxT = [ctx_p.tile([128, S], f32r, tag=f"c{k}", name=f"ctxT{k}")
                    for k in range(CT)]

            with tc.tile_pool(name="qkT", bufs=2) as qkT_p, \
                 tc.tile_pool(name="e", bufs=4) as e_p, \
                 tc.tile_pool(name="nrm", bufs=2) as nrm_p:

                def proj_si(w_t, x_t, bcol, hp, si, dst):
                    ps = proj_ps.tile([128, 512], f32, tag="pp")
                    for k in range(KT):
                        nc.tensor.matmul(
                            ps[:], w_t[k][:, ts(hp, 128)],
                            x_t[k][:, ts(si, 512)],
                            start=(k == 0), stop=(k == KT - 1))
                    nc.vector.tensor_scalar_add(
                        dst[:, ts(si, 512)], ps[:], bcol[:, hp:hp + 1])

                def proj_k(hp):
                    khT = qkT_p.tile([128, S], f32r, tag="khT")
                    for si in range(2):
                        proj_si(wk_t, xk_t, bkc, hp, si, khT)
                    return khT

                def proj_q(hp):
                    qhT = qkT_p.tile([128, S], f32r, tag="qhT")
                    for si in range(2):
                        proj_si(wq_t, xq_t, bqc, hp, si, qhT)
                    return qhT

                # prologue: only the si-half-0 projections gate the first
                # attention block; si-half-1 and all V chains weave into it
                khT = qkT_p.tile([128, S], f32r, tag="khT")
                proj_si(wk_t, xk_t, bkc, 0, 0, khT)
                qhT = qkT_p.tile([128, S], f32r, tag="qhT")
                proj_si(wq_t, xq_t, bqc, 0, 0, qhT)
                for hp in range(CT):
                    # attention for heads a=2*hp (partitions 0:64) and
                    # b=2*hp+1 (partitions 64:128), one 512-col q-half at
                    # a time; next head-pair's K/Q projections are issued
                    # between attention blocks so the PE stream stays dense
                    a, b = 2 * hp, 2 * hp + 1
                    khT_n = qhT_n = None
                    for qh in range(2):
                        psCa = ctx_ps.tile([128, 512], f32, tag="ca")
                        psCb = ctx_ps.tile([128, 512], f32, tag="cb")

                        def scores(t):
                            psS = scores_ps.tile([128, 1024], f32, tag="s")
                            nc.tensor.matmul(
                                psS[:, 0:512], khT[0:64, ts(t, 128)],
                                qhT[0:64, ts(qh, 512)], start=True, stop=True)
                            nc.tensor.matmul(
                                psS[:, 512:1024], khT[64:128, ts(t, 128)],
                                qhT[64:128, ts(qh, 512)], start=True,
                                stop=True, tile_position=(64, 0))
                            return psS

                        # software pipeline: PE issues scores(t+1) before
                        # PV(t) so it isn't parked waiting on exp(t).
                        # In the very first block (hp0,qh0) the V chains and
                        # the si-half-1 K/Q projections weave between the
                        # attention matmuls, filling the ACT-paced slack.
                        psS = scores(0)
                        for t in range(SKT):
                            eT = e_p.tile([128, 1024], f32r, tag="e")
                            nc.scalar.activation(eT[:], psS[:], EXP,
                                                 bias=maskb[t],
                                                 scale=1.0 / np.sqrt(HD))
                            if t + 1 < SKT:
                                psS = scores(t + 1)
                            if hp == 0 and qh == 0:
                                vchain(t)
                                if t == 1:
                                    proj_si(wk_t, xk_t, bkc, 0, 1, khT)
                                if t == 5:
                                    proj_si(wq_t, xq_t, bqc, 0, 1, qhT)
                            st, sp = (t == 0), (t == SKT - 1)
                            nc.tensor.matmul(
                                psCa[0:65, :], vnat[t][:, ts(a, 65)],
                                eT[:, 0:512], start=st, stop=sp)
                            nc.tensor.matmul(
                                psCb[0:65, :], vnat[t][:, ts(b, 65)],
                                eT[:, 512:1024], start=st, stop=sp)

                        if hp == 0:
                            if qh == 1:
                                khT_n = proj_k(1)
                                qhT_n = proj_q(1)
                        elif hp + 1 < CT:
                            if qh == 0:
                                khT_n = proj_k(hp + 1)
                            else:
                                qhT_n = proj_q(hp + 1)

                        for half, psC in ((0, psCa), (1, psCb)):
                            # PSUM->SBUF copies release the psC bank for the
                            # next accumulation chain immediately; the recip
                            # input must sit at base partition 0 (custom-DVE
                            # op quirk)
                            sc = nrm_p.tile([64, 512], f32, tag=f"s{half}")
                            nc.vector.tensor_copy(sc[:], psC[0:64, :])
                            den = nrm_p.tile([1, 512], f32, tag=f"d{half}")
                            nc.vector.tensor_copy(den[:], psC[64:65, :])
                            rec = nrm_p.tile([1, 512], f32, tag=f"r{half}")
                            nc.vector.reciprocal_approx_fast(
                                rec[:], den[:])
                            bc = nrm_p.tile([64, 512], f32, tag=f"b{half}")
                            nc.gpsimd.partition_broadcast(bc[:], rec[:])
                            nc.vector.tensor_mul(
                                ctxT[hp][64 * half:64 * half + 64,
                                         ts(qh, 512)],
                                sc[:], bc[:])

                    if hp + 1 < CT:
                        khT, qhT = khT_n, qhT_n

            # ---- output projection (partial: contraction over DC=512) ------
            # si-outer: the si=0 chains only read ctxT[:, 0:512], which is
            # complete after the qh=0 normalizations -- they overlap the
            # last head-pair's qh=1 attention
            with tc.tile_pool(name="outT", bufs=3) as out_p:
                for si in range(2):
                    for m in range(KT):
                        ps = proj_ps.tile([128, 512], f32, tag="pp")
                        for k in range(CT):
                            nc.tensor.matmul(
                                ps[:], wo_t[k][:, ts(m, 128)],
                                ctxT[k][:, ts(si, 512)],
                                start=(k == 0), stop=(k == CT - 1))
                        ot = out_p.tile([128, 512], f32, tag="o")
                        nc.scalar.copy(ot[:], ps[:])
                        nc.sync.dma_start(
                            out_d[ts(m, 128), ts(si, 512)], ot[:])

    nc.compile()
    return nc


def _get_compiled():
    global _COMPILED
    if _COMPILED is None:
        _COMPILED = _build()
    return _COMPILED


def _in_maps(q, k, v, mask, Wq, bq, Wk, bk, Wv, bv, Wo, bo):
    import ml_dtypes
    bf16 = ml_dtypes.bfloat16
    q = np.asarray(q, dtype=np.float32)
    k = np.asarray(k, dtype=np.float32)
    v = np.asarray(v, dtype=np.float32)
    mask = np.asarray(mask, dtype=np.int32)
    Wq = np.asarray(Wq, np.float32)
    Wk = np.asarray(Wk, np.float32)
    Wv = np.asarray(Wv, np.float32)
    Wo = np.asarray(Wo, np.float32)
    bq = np.asarray(bq, np.float32)
    bk = np.asarray(bk, np.float32)
    bv = np.asarray(bv, np.float32)

    qT = [np.ascontiguousarray(q[bi].T.astype(bf16)) for bi in range(B)]
    kT = [np.ascontiguousarray(k[bi].T.astype(bf16)) for bi in range(B)]
    vT = [np.ascontiguousarray(v[bi].T.astype(bf16)) for bi in range(B)]
    # mask bias columns: [128, SKT], col t = (mask[t*128:(t+1)*128]-1)*1e9
    mbias = [np.ascontiguousarray(
        ((mask[bi, 0].astype(np.float32) - 1.0) * 1e9)
        .reshape(SKT, 128).T) for bi in range(B)]
    whh = []
    for hh in range(2):
        sl = slice(hh * DC, (hh + 1) * DC)
        whh.append({
            "Wq": np.ascontiguousarray(Wq[sl, :].T.astype(bf16)),
            "Wk": np.ascontiguousarray(Wk[sl, :].T.astype(bf16)),
            "Wv": np.ascontiguousarray(Wv[sl, :].T.astype(bf16)),
            "Wo": np.ascontiguousarray(Wo[:, sl].T),
            "bqc": np.ascontiguousarray(bq[sl].reshape(CT, 128).T),
            "bkc": np.ascontiguousarray(bk[sl].reshape(CT, 128).T),
            "bv": np.ascontiguousarray(bv[sl]),
        })
    in_maps = []
    for c in range(NCORES):
        bidx, hh = c // 2, c % 2
        in_maps.append({
            "xq": qT[bidx],
            "xk": kT[bidx],
            "xv": vT[bidx],
            "maskbias": mbias[bidx],
            **whh[hh],
        })
    return in_maps


def _gather(results, bo):
    bo = np.asarray(bo, np.float32)
    out = np.empty((B, S, D), np.float32)
    for bidx in range(B):
        acc = results[2 * bidx]["out"] + results[2 * bidx + 1]["out"]
        out[bidx] = acc.T
        out[bidx] += bo
    return out


def kernel(q, k, v, mask, Wq, bq, Wk, bk, Wv, bv, Wo, bo, **_ignored):
    from concourse.bass_utils import run_bass_kernel_spmd

    nc = _get_compiled()
    in_maps = _in_maps(q, k, v, mask, Wq, bq, Wk, bk, Wv, bv, Wo, bo)
    res = run_bass_kernel_spmd(nc, in_maps, core_ids=list(range(NCORES)))
    return _gather(res.results, bo)


# revision 34
# speedup vs baseline: 1.0569x; 1.0569x over previous
"""Multi-head attention (B=4, S=1024, D=1024, H=16) on 8 Trainium2 NeuronCores.

Sharding (tensor-parallel over heads x data-parallel over batch):
core c handles batch b=c//2 and head-half hh=c%2 (8 of the 16 heads).
Each core projects Q/K/V for its 8 heads only (zero duplicated
projection FLOPs), runs attention for its 8 heads over the full
1024x1024 score matrix, and computes the PARTIAL output projection
ctx_half @ Wo[:, half].T.  The host sums the two partials per batch and
adds bo during the gather -- no device collectives.

Performance notes:
 - All device DMA is contiguous and coarse.  Tiny strided DMAs (ones
   columns, per-partition bias/mask columns) are eliminated: they
   generate thousands of 4-byte packets that monopolize the shared DMA
   engines for ~80us.  Mask/bias columns arrive host-packed as [128,n]
   tiles; the softmax ones-columns are written by a DVE tensor_scalar.
 - x and Wq/Wk/Wv stream in as bf16 (half the DMA bytes); everything
   downstream of the projections stays float32r (same 1 column/cycle PE
   streaming rate as bf16 at N=512, full fp32 precision).
 - Softmax reciprocal via reciprocal_approx_fast (~5x faster than the
   multipass DVE reciprocal).
 - Output-projection PSUM evacuation on the Scalar engine (idle after
   attention; DVE is the busier engine).
"""

import sys

for _p in ("/opt/trn_rl_repo", "/opt/pypackages"):
    if _p not in sys.path:
        sys.path.append(_p)

import numpy as np

B = 4
S = 1024
D = 1024
H = 16
HD = 64
HC = 8            # heads per core
DC = HC * HD      # 512 projection dims per core
KT = D // 128     # 8 contraction tiles (model dim)
CT = DC // 128    # 4 contraction tiles (core's ctx dims) = head pairs
SKT = S // 128    # 8 key tiles
NCORES = 8

_COMPILED = None


def _build():
    import concourse.bass as bass
    import concourse.mybir as mybir
    from concourse import bacc
    from concourse.bass import ts
    from concourse.tile import TileContext

    f32 = mybir.dt.float32
    f32r = mybir.dt.float32r
    bf16 = mybir.dt.bfloat16
    EXP = mybir.ActivationFunctionType.Exp

    nc = bacc.Bacc("TRN2", target_bir_lowering=False, debug=False,
                   num_devices=NCORES)

    # activations/weights pre-transposed + contiguous; x and W in bf16
    xq_d = nc.dram_tensor("xq", [D, S], bf16, kind="ExternalInput")
    xk_d = nc.dram_tensor("xk", [D, S], bf16, kind="ExternalInput")
    xv_d = nc.dram_tensor("xv", [D, S], bf16, kind="ExternalInput")
    wq_d = nc.dram_tensor("Wq", [D, DC], bf16, kind="ExternalInput")
    wk_d = nc.dram_tensor("Wk", [D, DC], bf16, kind="ExternalInput")
    wv_d = nc.dram_tensor("Wv", [D, DC], bf16, kind="ExternalInput")
    wo_d = nc.dram_tensor("Wo", [DC, D], f32, kind="ExternalInput")
    # host-packed per-partition columns: mask bias [128,SKT], biases [128,CT]
    mb_d = nc.dram_tensor("maskbias", [128, SKT], f32, kind="ExternalInput")
    bq_d = nc.dram_tensor("bqc", [128, CT], f32, kind="ExternalInput")
    bk_d = nc.dram_tensor("bkc", [128, CT], f32, kind="ExternalInput")
    bv_d = nc.dram_tensor("bv", [DC], f32, kind="ExternalInput")
    out_d = nc.dram_tensor("out", [D, S], f32, kind="ExternalOutput")

    with TileContext(nc) as tc:
        from contextlib import ExitStack
        with ExitStack() as stack:
            const = stack.enter_context(tc.tile_pool(name="const", bufs=1))
            vnat_p = stack.enter_context(tc.tile_pool(name="vnat", bufs=1))
            ctx_p = stack.enter_context(tc.tile_pool(name="ctxT", bufs=1))
            xq_p = stack.enter_context(tc.tile_pool(name="xq", bufs=1))
            xk_p = stack.enter_context(tc.tile_pool(name="xk", bufs=1))
            xv_p = stack.enter_context(tc.tile_pool(name="xv", bufs=1))
            w_p = stack.enter_context(tc.tile_pool(name="w", bufs=1))
            proj_ps = stack.enter_context(
                tc.tile_pool(name="proj_ps", bufs=2, space="PSUM"))
            scores_ps = stack.enter_context(
                tc.tile_pool(name="scores_ps", bufs=2, space="PSUM"))
            ctx_ps = stack.enter_context(
                tc.tile_pool(name="ctx_ps", bufs=1, space="PSUM"))

            # ---- small host-packed constants first (complete in ~1us,
            # unblock the projection bias-adds) ------------------------------
            mb_sb = const.tile([128, SKT], f32, tag="mb")
            nc.sync.dma_start(mb_sb[:], mb_d[:])
            maskb = [mb_sb[:, t:t + 1] for t in range(SKT)]
            bqc = const.tile([128, CT], f32, tag="bqc")
            nc.sync.dma_start(bqc[:], bq_d[:])
            bkc = const.tile([128, CT], f32, tag="bkc")
            nc.sync.dma_start(bkc[:], bk_d[:])
            bv_bc = const.tile([128, DC], f32, tag="bvbc")
            nc.sync.dma_start(
                bv_bc[:],
                bass.AP(tensor=bv_d, offset=0, ap=[[0, 128], [1, DC]]))

            # ---- big streaming loads, in consumption order.  V/K inputs
            # stream on the sync HWDGE queues, Q/O inputs on the gpsimd
            # SWDGE queues (separate DMA engines -> parallel streams).
            # Weight/activation k-tiles interleave so accumulation chain
            # k can start as soon as pair k has landed. -------------------
            xv_t, wv_t, xk_t, wk_t, xq_t, wq_t = [], [], [], [], [], []
            # two independent DMA streams (sync HWDGE / gpsimd SWDGE);
            # V-phase (w,x) pairs alternate between them so consecutive
            # accumulation-chain steps land twice as fast
            eng = [nc.sync, nc.gpsimd]
            for k in range(KT):
                e = eng[k % 2]
                t = w_p.tile([128, DC], bf16, tag=f"wv{k}")
                if k < 2:
                    # chunk early loads across queues: a whole tile on one
                    # queue is ~8us (per-DMA-engine ~32GB/s); chunks land
                    # in parallel and un-gate the first V chains
                    e.dma_start(t[:, 0:256], wv_d[ts(k, 128), 0:256])
                    e.dma_start(t[:, 256:512], wv_d[ts(k, 128), 256:512])
                else:
                    e.dma_start(t[:], wv_d[ts(k, 128), :])
                wv_t.append(t)
                t = xv_p.tile([128, S], bf16, tag=f"xv{k}")
                nch = 4 if k < 2 else 2
                for c in range(nch):
                    w = S // nch
                    e.dma_start(t[:, c * w:(c + 1) * w],
                                xv_d[ts(k, 128), c * w:(c + 1) * w])
                xv_t.append(t)
            for k in range(KT):
                e = eng[k % 2]
                t = w_p.tile([128, DC], bf16, tag=f"wk{k}")
                e.dma_start(t[:], wk_d[ts(k, 128), :])
                wk_t.append(t)
                t = xk_p.tile([128, S], bf16, tag=f"xk{k}")
                e.dma_start(t[:, 0:512], xk_d[ts(k, 128), 0:512])
                xk_t.append(t)
            for k in range(KT):
                eng[k % 2].dma_start(xk_t[k][:, 512:1024],
                                     xk_d[ts(k, 128), 512:1024])
            for k in range(KT):
                e = eng[k % 2]
                t = w_p.tile([128, DC], bf16, tag=f"wq{k}")
                e.dma_start(t[:], wq_d[ts(k, 128), :])
                wq_t.append(t)
                t = xq_p.tile([128, S], bf16, tag=f"xq{k}")
                e.dma_start(t[:, 0:512], xq_d[ts(k, 128), 0:512])
                xq_t.append(t)
            for k in range(KT):
                eng[k % 2].dma_start(xq_t[k][:, 512:1024],
                                     xq_d[ts(k, 128), 512:1024])
            wo_t = []
            for k in range(CT):
                t = w_p.tile([128, D], f32r, tag=f"wo{k}")
                eng[k % 2].dma_start(t[:], wo_d[ts(k, 128), :].bitcast(f32r))
                wo_t.append(t)

            # preload the exp activation table during the startup window so
            # the first real exp doesn't pay the ~2.7us table switch
            warm = const.tile([128, 1], f32, tag="warm")
            nc.scalar.activation(warm[:], mb_sb[:, 0:1], EXP, scale=1.0)

            # ---- V projection (natural layout: [sk, dout] + ones col) ------
            vnat = [vnat_p.tile([128, HC * 65], f32r, tag=f"v{m}",
                                name=f"vnat{m}")
                    for m in range(SKT)]
            bv3 = bv_bc[:].rearrange("p (a b) -> p a b", b=1)
            for m in range(SKT):
                vv = vnat[m][:].rearrange("p (h x) -> p h x", x=65)
                # ones column: in*0 + 1 from a known-finite source (DVE
                # rounds to f32r; memset can't produce f32r)
                nc.vector.tensor_scalar(vv[:, :, 64:65], bv3[:, 0:HC, :],
                                        0.0, 1.0,
                                        mybir.AluOpType.mult,
                                        mybir.AluOpType.add)
                ps = proj_ps.tile([128, 512], f32, tag="pp")
                for k in range(KT):
                    nc.tensor.matmul(
                        ps[:], xv_t[k][:, ts(m, 128)], wv_t[k][:],
                        start=(k == 0), stop=(k == KT - 1))
                nc.vector.tensor_add(
                    vv[:, :, 0:64],
                    ps[:].rearrange("p (h x) -> p h x", x=64),
                    bv_bc[:].rearrange("p (h x) -> p h x", x=64))

            # ---- per head-pair: K/Q projection + attention -----------------
            ctxT = [ctx_p.tile([128, S], f32r, tag=f"c{k}", name=f"ctxT{k}")
                    for k in range(CT)]

            with tc.tile_pool(name="qkT", bufs=2) as qkT_p, \
                 tc.tile_pool(name="e", bufs=4) as e_p, \
                 tc.tile_pool(name="nrm", bufs=2) as nrm_p:

                def proj_k(hp):
                    khT = qkT_p.tile([128, S], f32r, tag="khT")
                    for si in range(2):
                        ps = proj_ps.tile([128, 512], f32, tag="pp")
                        for k in range(KT):
                            nc.tensor.matmul(
                                ps[:], wk_t[k][:, ts(hp, 128)],
                                xk_t[k][:, ts(si, 512)],
                                start=(k == 0), stop=(k == KT - 1))
                        nc.vector.tensor_scalar_add(
                            khT[:, ts(si, 512)], ps[:], bkc[:, hp:hp + 1])
                    return khT

                def proj_q(hp):
                    qhT = qkT_p.tile([128, S], f32r, tag="qhT")
                    for si in range(2):
                        ps = proj_ps.tile([128, 512], f32, tag="pp")
                        for k in range(KT):
                            nc.tensor.matmul(
                                ps[:], wq_t[k][:, ts(hp, 128)],
                                xq_t[k][:, ts(si, 512)],
                                start=(k == 0), stop=(k == KT - 1))
                        nc.vector.tensor_scalar_add(
                            qhT[:, ts(si, 512)], ps[:], bqc[:, hp:hp + 1])
                    return qhT

                khT = proj_k(0)
                qhT = proj_q(0)
                for hp in range(CT):
                    # attention for heads a=2*hp (partitions 0:64) and
                    # b=2*hp+1 (partitions 64:128), one 512-col q-half at
                    # a time; next head-pair's K/Q projections are issued
                    # between attention blocks so the PE stream stays dense
                    a, b = 2 * hp, 2 * hp + 1
                    khT_n = qhT_n = None
                    for qh in range(2):
                        psCa = ctx_ps.tile([128, 512], f32, tag="ca")
                        psCb = ctx_ps.tile([128, 512], f32, tag="cb")

                        def scores(t):
                            psS = scores_ps.tile([128, 1024], f32, tag="s")
                            nc.tensor.matmul(
                                psS[:, 0:512], khT[0:64, ts(t, 128)],
                                qhT[0:64, ts(qh, 512)], start=True, stop=True)
                            nc.tensor.matmul(
                                psS[:, 512:1024], khT[64:128, ts(t, 128)],
                                qhT[64:128, ts(qh, 512)], start=True,
                                stop=True, tile_position=(64, 0))
                            return psS

                        # software pipeline: PE issues scores(t+1) before
                        # PV(t) so it isn't parked waiting on exp(t)
                        psS = scores(0)
                        for t in range(SKT):
                            eT = e_p.tile([128, 1024], f32r, tag="e")
                            nc.scalar.activation(eT[:], psS[:], EXP,
                                                 bias=maskb[t],
                                                 scale=1.0 / np.sqrt(HD))
                            if t + 1 < SKT:
                                psS = scores(t + 1)
                            st, sp = (t == 0), (t == SKT - 1)
                            nc.tensor.matmul(
                                psCa[0:65, :], vnat[t][:, ts(a, 65)],
                                eT[:, 0:512], start=st, stop=sp)
                            nc.tensor.matmul(
                                psCb[0:65, :], vnat[t][:, ts(b, 65)],
                                eT[:, 512:1024], start=st, stop=sp)

                        if hp + 1 < CT:
                            if qh == 0:
                                khT_n = proj_k(hp + 1)
                            else:
                                qhT_n = proj_q(hp + 1)

                        for half, psC in ((0, psCa), (1, psCb)):
                            # PSUM->SBUF copies release the psC bank for the
                            # next accumulation chain immediately; the recip
                            # input must sit at base partition 0 (custom-DVE
                            # op quirk)
                            sc = nrm_p.tile([64, 512], f32, tag=f"s{half}")
                            nc.vector.tensor_copy(sc[:], psC[0:64, :])
                            den = nrm_p.tile([1, 512], f32, tag=f"d{half}")
                            nc.vector.tensor_copy(den[:], psC[64:65, :])
                            rec = nrm_p.tile([1, 512], f32, tag=f"r{half}")
                            nc.vector.reciprocal_approx_fast(
                                rec[:], den[:])
                            bc = nrm_p.tile([64, 512], f32, tag=f"b{half}")
                            nc.gpsimd.partition_broadcast(bc[:], rec[:])
                            nc.vector.tensor_mul(
                                ctxT[hp][64 * half:64 * half + 64,
                                         ts(qh, 512)],
                                sc[:], bc[:])

                    if hp + 1 < CT:
                        khT, qhT = khT_n, qhT_n

            # ---- output projection (partial: contraction over DC=512) ------
            # si-outer: the si=0 chains only read ctxT[:, 0:512], which is
            # complete after the qh=0 normalizations -- they overlap the
            # last head-pair's qh=1 attention
            with tc.tile_pool(name="outT", bufs=3) as out_p:
                for si in range(2):
                    for m in range(KT):
                        ps = proj_ps.tile([128, 512], f32, tag="pp")
                        for k in range(CT):
                            nc.tensor.matmul(
                                ps[:], wo_t[k][:, ts(m, 128)],
                                ctxT[k][:, ts(si, 512)],
                                start=(k == 0), stop=(k == CT - 1))
                        ot = out_p.tile([128, 512], f32, tag="o")
                        nc.scalar.copy(ot[:], ps[:])
                        nc.sync.dma_start(
                            out_d[ts(m, 128), ts(si, 512)], ot[:])

    nc.compile()
    return nc


def _get_compiled():
    global _COMPILED
    if _COMPILED is None:
        _COMPILED = _build()
    return _COMPILED


def _in_maps(q, k, v, mask, Wq, bq, Wk, bk, Wv, bv, Wo, bo):
    import ml_dtypes
    bf16 = ml_dtypes.bfloat16
    q = np.asarray(q, dtype=np.float32)
    k = np.asarray(k, dtype=np.float32)
    v = np.asarray(v, dtype=np.float32)
    mask = np.asarray(mask, dtype=np.int32)
    Wq = np.asarray(Wq, np.float32)
    Wk = np.asarray(Wk, np.float32)
    Wv = np.asarray(Wv, np.float32)
    Wo = np.asarray(Wo, np.float32)
    bq = np.asarray(bq, np.float32)
    bk = np.asarray(bk, np.float32)
    bv = np.asarray(bv, np.float32)

    qT = [np.ascontiguousarray(q[bi].T.astype(bf16)) for bi in range(B)]
    kT = [np.ascontiguousarray(k[bi].T.astype(bf16)) for bi in range(B)]
    vT = [np.ascontiguousarray(v[bi].T.astype(bf16)) for bi in range(B)]
    # mask bias columns: [128, SKT], col t = (mask[t*128:(t+1)*128]-1)*1e9
    mbias = [np.ascontiguousarray(
        ((mask[bi, 0].astype(np.float32) - 1.0) * 1e9)
        .reshape(SKT, 128).T) for bi in range(B)]
    whh = []
    for hh in range(2):
        sl = slice(hh * DC, (hh + 1) * DC)
        whh.append({
            "Wq": np.ascontiguousarray(Wq[sl, :].T.astype(bf16)),
            "Wk": np.ascontiguousarray(Wk[sl, :].T.astype(bf16)),
            "Wv": np.ascontiguousarray(Wv[sl, :].T.astype(bf16)),
            "Wo": np.ascontiguousarray(Wo[:, sl].T),
            "bqc": np.ascontiguousarray(bq[sl].reshape(CT, 128).T),
            "bkc": np.ascontiguousarray(bk[sl].reshape(CT, 128).T),
            "bv": np.ascontiguousarray(bv[sl]),
        })
    in_maps = []
    for c in range(NCORES):
        bidx, hh = c // 2, c % 2
        in_maps.append({
            "xq": qT[bidx],
            "xk": kT[bidx],
            "xv": vT[bidx],
            "maskbias": mbias[bidx],
            **whh[hh],
        })
    return in_maps


def _gather(results, bo):
    bo = np.asarray(bo, np.float32)
    out = np.empty((B, S, D), np.float32)
    for bidx in range(B):
        acc = results[2 * bidx]["out"] + results[2 * bidx + 1]["out"]
        out[bidx] = acc.T
        out[bidx] += bo
    return out


def kernel(q, k, v, mask, Wq, bq, Wk, bk, Wv, bv, Wo, bo, **_ignored):
    from concourse.bass_utils import run_bass_kernel_spmd

    nc = _get_compiled()
    in_maps = _in_maps(q, k, v, mask, Wq, bq, Wk, bk, Wv, bv, Wo, bo)
    res = run_bass_kernel_spmd(nc, in_maps, core_ids=list(range(NCORES)))
    return _gather(res.results, bo)
